# revision 7
# baseline (speedup 1.0000x reference)
"""Causal self-attention (B=4, T=2048, C=1024, H=16) on 8 TRN2 NeuronCores.

Sharding: 8 cores = 4 batches x 2 head-groups (Megatron tensor-parallel over
heads + data-parallel over batch). Each core runs a software-pipelined
three-stage kernel over 512-token query tiles:

  stage 1: qkv projection (bf16): q/k written pair-packed [head_dim on
           partitions, T free], V token-major with an appended ones-column
           per head (softmax sums ride along in PV).
  stage 2: causal attention (bf16), computed TRANSPOSED (S^T = K^T x Q, keys
           on PSUM partitions, queries free). Pair-packed 64-contractions via
           tile_position row groups; causal triangle accumulated into PSUM
           with a bf16 identity matmul on diagonal j-tiles; exp on the Scalar
           engine straight out of PSUM (no max-subtraction: logits are O(1));
           PV with ones-augmented V yields unnormalized y^T plus softmax
           sums. Normalization: DVE reciprocal of the sums rows, broadcast
           down 64 partitions with a K=1 PE matmul, DVE multiplies.
  stage 3: output projection (bf16); psum->sbuf copies on the Scalar engine,
           output DMAs issued from the (otherwise idle) Pool engine.

The stages are interleaved so no engine idles across stage boundaries: while
ACT exps attention tile t, the PE runs qkv projection for tile t+1 and output
projection for tile t-1 as filler work between attention j-iterations.

Host: shards inputs, sums the two partial outputs per batch, adds b_proj and
the folded v-bias term (y = softmax@v + b_v exactly, so w_proj @ b_v moves
into the output bias).
"""

import sys

if "/opt/trn_rl_repo" not in sys.path:
    sys.path.insert(0, "/opt/trn_rl_repo")

from collections import deque
from contextlib import ExitStack

import numpy as np
import ml_dtypes

import concourse.bass as bass
import concourse.tile as tile
from concourse import bacc, mybir
from concourse.bass_utils import run_bass_kernel_spmd

F32 = mybir.dt.float32
F32R = mybir.dt.float32r
BF16 = mybir.dt.bfloat16
AF = mybir.ActivationFunctionType
MUL = mybir.AluOpType.mult
ADD = mybir.AluOpType.add

B, T, C = 4, 2048, 1024
H, HD = 16, 64
NHL = 8          # heads per core (local)
NPAIR = 4        # head pairs per core
P = 128
TQ = 512         # query tile (free dim)
TJ = 128         # key tile (partitions)
NIT = T // TQ    # 4 query tiles
NTS = T // P     # 16 token sub-tiles
NCT = C // P     # 8 contraction tiles over C
NEG = -100000.0  # additive causal mask value


def build_kernel():
    nc = bacc.Bacc("TRN2", target_bir_lowering=False)

    xb = nc.declare_dram_parameter("xb", [P, NCT, T], BF16, isOutput=False)
    wqk = nc.declare_dram_parameter("wqk", [P, NCT, 1024], BF16, isOutput=False)
    wv = nc.declare_dram_parameter("wv", [P, NCT, 512], BF16, isOutput=False)
    wp = nc.declare_dram_parameter("wp", [P, NPAIR, 1024], BF16, isOutput=False)
    bqk = nc.declare_dram_parameter("bqk", [P, 8], F32, isOutput=False)
    tri = nc.declare_dram_parameter("tri", [P, P], BF16, isOutput=False)
    idn = nc.declare_dram_parameter("idn", [P, P], BF16, isOutput=False)
    out = nc.declare_dram_parameter("out", [T, C], BF16, isOutput=True)

    with tile.TileContext(nc) as tc, ExitStack() as ctx:
        ctx.enter_context(
            nc.allow_low_precision(
                reason="bf16 value path validated against the fp32 reference"
            )
        )
        persist = ctx.enter_context(tc.tile_pool(name="persist", bufs=1))
        xbp = ctx.enter_context(tc.tile_pool(name="xbp", bufs=2))
        s2att = ctx.enter_context(tc.tile_pool(name="s2att", bufs=4))
        s2n = ctx.enter_context(tc.tile_pool(name="s2n", bufs=2))
        s3o = ctx.enter_context(tc.tile_pool(name="s3o", bufs=4))
        qkps = ctx.enter_context(tc.tile_pool(name="qkps", bufs=2, space="PSUM"))
        pvps = ctx.enter_context(tc.tile_pool(name="pvps", bufs=3, space="PSUM"))
        s13ps = ctx.enter_context(tc.tile_pool(name="s13ps", bufs=1, space="PSUM"))

        q_sb = persist.tile([P, NPAIR, T], BF16)
        k_sb = persist.tile([P, NPAIR, T], BF16)
        v_sb = persist.tile([P, NTS, NHL, HD + 1], BF16)
        y_sb = persist.tile([P, NPAIR, T], BF16)
        wqk_sb = persist.tile([P, NCT, 1024], BF16)
        wv_sb = persist.tile([P, NCT, 512], BF16)
        wp_sb = persist.tile([P, NPAIR, 1024], BF16)
        bqk_sb = persist.tile([P, 8], F32)
        tri_sb = persist.tile([P, P], BF16)
        idn_sb = persist.tile([P, P], BF16)
        ones_sb = persist.tile([P, HD], F32R)

        nc.sync.dma_start(bqk_sb, bqk[:])
        nc.sync.dma_start(tri_sb, tri[:])
        nc.sync.dma_start(idn_sb, idn[:])
        nc.vector.memset(ones_sb[64:65, :].bitcast(F32), 1.0)
        # ones columns of the augmented V (softmax sums ride along in PV)
        nc.vector.memset(v_sb[:, :, :, HD : HD + 1], 1.0)

        xb_tiles = {}

        def dma_xb(t):
            xt = xbp.tile([P, NCT, TQ], BF16, tag="xb")
            nc.sync.dma_start(xt, xb[:, :, t * TQ : (t + 1) * TQ])
            xb_tiles[t] = xt

        def s1_qk_unit(t, m, ps_pool=None):
            """qkv projection for q/k output block m (0-3 q, 4-7 k)."""
            pool, tag = ps_pool or (s13ps, "ps")
            ps = pool.tile([P, TQ], F32, tag=tag)
            xt = xb_tiles[t]
            for c in range(NCT):
                nc.tensor.matmul(
                    ps,
                    wqk_sb[:, c, m * P : (m + 1) * P],
                    xt[:, c, :],
                    start=(c == 0),
                    stop=(c == NCT - 1),
                )
            dst = q_sb if m < 4 else k_sb
            nc.vector.tensor_scalar_add(
                dst[:, m % 4, t * TQ : (t + 1) * TQ], ps, bqk_sb[:, m : m + 1]
            )

        def s1_v_unit(t, s, ps_pool=None):
            """v projection for token subtile s of query tile t."""
            pool, tag = ps_pool or (s13ps, "ps")
            ps = pool.tile([P, 512], F32, tag=tag)
            xt = xb_tiles[t]
            for c in range(NCT):
                nc.tensor.matmul(
                    ps,
                    xt[:, c, s * P : (s + 1) * P],
                    wv_sb[:, c, :],
                    start=(c == 0),
                    stop=(c == NCT - 1),
                )
            tsub = t * (TQ // P) + s
            nc.vector.tensor_copy(
                v_sb[:, tsub, :, 0:HD],
                ps.rearrange("p (h d) -> p h d", h=NHL),
            )

        def s3_unit(tt, ot):
            """output projection for token subtile tt, output half ot."""
            ps = s13ps.tile([P, 512], F32, tag="ps")
            for a in range(NPAIR):
                nc.tensor.matmul(
                    ps,
                    y_sb[:, a, tt * P : (tt + 1) * P],
                    wp_sb[:, a, ot * 512 : (ot + 1) * 512],
                    start=(a == 0),
                    stop=(a == NPAIR - 1),
                )
            osb = s3o.tile([P, 512], BF16, tag="osb")
            nc.scalar.copy(osb, ps)
            nc.gpsimd.dma_start(
                out[tt * P : (tt + 1) * P, ot * 512 : (ot + 1) * 512], osb
            )

        def attn_block(a, it, pop_filler):
            i0 = it * TQ
            njt = (i0 + TQ) // TJ
            pv = [
                pvps.tile([P, TQ], F32, tag="pv", name=f"pv0_{a}_{it}"),
                pvps.tile([P, TQ], F32, tag="pv", name=f"pv1_{a}_{it}"),
            ]
            for jt in range(njt):
                pop_filler()
                j0 = jt * TJ
                d = j0 - i0
                istart = max(d, 0)
                nn = TQ - istart
                qk = qkps.tile([P, 2, TQ], F32, tag="qk")
                for e in (0, 1):
                    nc.tensor.matmul(
                        qk[:, e, istart:TQ],
                        k_sb[64 * e : 64 * e + 64, a, j0 : j0 + TJ],
                        q_sb[64 * e : 64 * e + 64, a, i0 + istart : i0 + TQ],
                        start=True,
                        stop=(d < 0),
                        tile_position=(64 * e, 0),
                    )
                    if d >= 0:
                        nc.tensor.matmul(
                            qk[:, e, istart : istart + TJ],
                            idn_sb,
                            tri_sb,
                            start=False,
                            stop=True,
                            tile_position=(0, 0),
                        )
                att = s2att.tile([P, 2, TQ], BF16, tag="att")
                nc.scalar.activation(att[:, :, 0:nn], qk[:, :, istart:TQ], AF.Exp)
                for e in (0, 1):
                    nc.tensor.matmul(
                        pv[e][0 : HD + 1, istart:TQ],
                        v_sb[:, jt, 2 * a + e, :],
                        att[:, e, 0:nn],
                        start=(jt == 0),
                        stop=(jt == njt - 1),
                    )
            # normalize: recip of sums rows, PE-broadcast down 64 partitions,
            # DVE multiplies (e=1 lands at partitions 64:128 via an sb-sb DMA
            # because DVE cannot shift partition base).
            rt = s2n.tile([P, 2, TQ], F32R, tag="rt")
            for e in (0, 1):
                nc.vector.reciprocal(rt[HD : HD + 1, e, :], pv[e][HD : HD + 1, :])
            rb = qkps.tile([P, 2, TQ], F32, tag="qk")
            for e in (0, 1):
                nc.tensor.matmul(
                    rb[0:HD, e, :],
                    ones_sb[64:65, :],
                    rt[HD : HD + 1, e, :],
                    start=True,
                    stop=True,
                    tile_position=(64, 0),
                )
            rbs = s2n.tile([HD, 2, TQ], F32R, tag="rbs")
            nc.vector.tensor_copy(rbs, rb[0:HD, :, :])
            nc.vector.tensor_mul(
                y_sb[0:HD, a, i0 : i0 + TQ], pv[0][0:HD, :], rbs[:, 0, :]
            )
            yt = s2n.tile([HD, TQ], BF16, tag="yt")
            nc.vector.tensor_mul(yt, pv[1][0:HD, :], rbs[:, 1, :])
            nc.gpsimd.dma_start(y_sb[64:128, a, i0 : i0 + TQ], yt)

        # ---------------- prologue: stage 1 for tile 0 ----------------
        dma_xb(0)
        for c in range(NCT):
            nc.sync.dma_start(wqk_sb[:, c, :], wqk[:, c, :])
            nc.sync.dma_start(wv_sb[:, c, :], wv[:, c, :])
        # rotate psum across three pools (s13ps + 3 pvps bufs are all free
        # here) so the prologue pipelines instead of ping-ponging one bank
        pro_pools = [(s13ps, "ps"), (pvps, "pv"), (pvps, "pv"), (pvps, "pv")]
        for m in range(8):
            s1_qk_unit(0, m, ps_pool=pro_pools[m % 4])
        for s in range(4):
            s1_v_unit(0, s, ps_pool=pro_pools[s % 4])
        nc.sync.dma_start(wp_sb, wp[:])

        # ---------------- pipelined main loop ----------------
        for t in range(NIT):
            fillers = deque()
            if t + 1 < NIT:
                dma_xb(t + 1)
                for m in range(8):
                    fillers.append(lambda t=t, m=m: s1_qk_unit(t + 1, m))
                for s in range(4):
                    fillers.append(lambda t=t, s=s: s1_v_unit(t + 1, s))
            if t > 0:
                for tt in range((t - 1) * 4, t * 4):
                    for ot in range(2):
                        fillers.append(lambda tt=tt, ot=ot: s3_unit(tt, ot))

            iters_total = 4 * 4 * (t + 1)
            units_total = len(fillers)
            state = {"iter": 0, "done": 0}

            def pop_filler():
                state["iter"] += 1
                target = (units_total * state["iter"] + iters_total - 1) // iters_total
                while fillers and state["done"] < target:
                    fillers.popleft()()
                    state["done"] += 1

            for a in range(NPAIR):
                attn_block(a, t, pop_filler)
            while fillers:
                fillers.popleft()()

        # ---------------- epilogue: last output tile ----------------
        for tt in range(12, 16):
            for ot in range(2):
                s3_unit(tt, ot)

    nc.compile()
    return nc


_NC_CACHE = None


def _get_nc():
    global _NC_CACHE
    if _NC_CACHE is None:
        _NC_CACHE = build_kernel()
    return _NC_CACHE


def _shard_inputs(x, w_qkv, b_qkv, w_proj):
    """Build the 8 per-core input maps. Core id = 2*batch + head_group."""
    bf16 = ml_dtypes.bfloat16
    tri_np = np.where(
        np.arange(P)[None, :] >= np.arange(P)[:, None], 0.0, NEG
    ).astype(bf16)
    idn_np = np.eye(P, dtype=bf16)

    wqk_arrs, wv_arrs, wp_arrs, bqk_arrs = [], [], [], []
    for g in range(2):
        s = slice(g * 512, (g + 1) * 512)
        wqk_full = np.concatenate(
            [w_qkv[0:1024][s] / 8.0, w_qkv[1024:2048][s]], axis=0
        )  # [1024 f, 1024 c]
        wqk_arrs.append(
            np.ascontiguousarray(
                wqk_full.T.reshape(NCT, P, 1024).transpose(1, 0, 2)
            ).astype(bf16)
        )
        bqk_full = np.concatenate([b_qkv[0:1024][s] / 8.0, b_qkv[1024:2048][s]])
        bqk_arrs.append(
            np.ascontiguousarray(bqk_full.reshape(8, P).T).astype(np.float32)
        )
        wv_rows = w_qkv[2048:3072][s]  # [512 f, 1024 c]
        wv_arrs.append(
            np.ascontiguousarray(
                wv_rows.T.reshape(NCT, P, 512).transpose(1, 0, 2)
            ).astype(bf16)
        )
        wp_rhs = w_proj[:, s].T  # [512 hd, 1024 o]
        wp_arrs.append(
            np.ascontiguousarray(
                wp_rhs.reshape(NPAIR, P, 1024).transpose(1, 0, 2)
            ).astype(bf16)
        )

    in_maps = []
    for b in range(B):
        xt = np.ascontiguousarray(
            x[b].T.reshape(NCT, P, T).transpose(1, 0, 2)
        ).astype(bf16)
        for g in range(2):
            in_maps.append(
                {
                    "xb": xt,
                    "wqk": wqk_arrs[g],
                    "wv": wv_arrs[g],
                    "wp": wp_arrs[g],
                    "bqk": bqk_arrs[g],
                    "tri": tri_np,
                    "idn": idn_np,
                }
            )
    return in_maps


def kernel(x, w_qkv, b_qkv, w_proj, b_proj, _trace=False, _trace_kwargs=None):
    x = np.asarray(x, dtype=np.float32)
    w_qkv = np.asarray(w_qkv, dtype=np.float32)
    b_qkv = np.asarray(b_qkv, dtype=np.float32)
    w_proj = np.asarray(w_proj, dtype=np.float32)
    b_proj = np.asarray(b_proj, dtype=np.float32)

    nc = _get_nc()
    in_maps = _shard_inputs(x, w_qkv, b_qkv, w_proj)
    res = run_bass_kernel_spmd(
        nc, in_maps, core_ids=list(range(8)), trace=_trace,
        **(_trace_kwargs or {}),
    )
    # v-bias folds exactly through softmax: y = softmax@v + b_v
    b_eff = b_proj + w_proj @ b_qkv[2048:3072]
    out = np.empty((B, T, C), np.float32)
    for b in range(B):
        out[b] = (
            res.results[2 * b]["out"].astype(np.float32)
            + res.results[2 * b + 1]["out"].astype(np.float32)
            + b_eff
        )
    if _trace:
        return out, res
    return out
